# revision 66
# baseline (speedup 1.0000x reference)
"""Trainium2 Bass kernel for AttentionPooling.

Math (per batch element b):
  xf = x[b] reshaped [C, N] with C=512, N=4096
  q = wq@xf + bq ; k = wk@xf + bk ; v = wv@xf + bv          (each [64, N])
  logits = q @ k^T  [64, 64];  attn = softmax(logits, axis over rows o)
  out[b] = mean_n(attn @ v) = attn @ mean_n(v)              ([64])

Two algebraic collapses make most of the reference FLOPs vanish:
  1. attn does not depend on n, so mean_n(attn @ v) = attn @ vbar with
     vbar = mean_n(v) — the heavy [64, N] attn@v product becomes a [64]
     vector product.
  2. vbar = wv @ mean_n(x) + bv, so v is NEVER projected per-pixel: the
     [64, N] v-projection becomes a free-dim reduction of x (done on the
     otherwise-idle Vector/Scalar engines) plus a tiny [64,512]x[512]
     matmul.  Only q and k are projected per-pixel.

Implementation, per batch element:
  - x, wq, wk, wv are shipped as fp16 (10-bit mantissa, same class as
    tf32/f32r device rounding; empirically 4.5e-3 end-to-end rel err)
    which halves the HBM traffic for x — the dominant memory cost.  x is
    DMAd one [128, 4096] tile per C-chunk: full 8KB-contiguous rows,
    which measure ~25% faster aggregate than 4KB descriptors under
    8-core HBM contention (~323 vs ~260 GB/s per core).
  - X-STATIONARY fused projection: each [128, 128] x chunk-subtile is
    the matmul weight; ONE matmul per (chunk, subtile) streams the fused
    [wqT | wkT] 128-column block (FWL-eligible) through it, producing
    [q0T | k0T] directly in n-major layout (no transposes), fp32 PSUM
    accumulation over the 4 C-chunks.
  - One 3D-AP vector-engine copy per projection tile moves all 4
    subtiles' [1 | qT | kT | 1] attention operands to SBUF (the ones
    columns are pre-written once per pool slot); the ones-augmented
    [65, 65] attention matmuls (lhsT=[kT|1], rhs=[1|qT], accumulated
    over all 32 n-subtiles) produce L0^T AND sum_n k0 (column 0) and
    sum_n q0 (partition-64 row) — the bias-correction sums ride the
    same accumulation.  They run one projection tile behind (and the
    last tile's attention flushes at the NEXT batch's first tile) so
    the in-order PE queue never waits on a DVE copy.
  - sum_n x is split across three engines to stay under the DMA floor:
    two fp16 2x-mode binary-tree add levels on the DVE (tiles 0-3), two
    more levels on the otherwise-idle GPSIMD as pair-fused [128,2,*]
    ops (tiles 2-3), and one small f32 reduce + fp16 cast back on the
    DVE, deferred into the next batch's tile 0 so the slow GPSIMD fold
    never head-of-line blocks the DVE queue.
  - Bias corrections applied analytically on the 64x64 logits:
      L^T = L0^T + bq (x) (sk + N bk) + bk (x) sq
    (valid because logits(q0+bq, k0+bk) is bilinear and attn/softmax only
    needs the full L^T).
  - The finalize chain staggers one stage per tile of the next batch
    (ti0 attn-flush+sq_sb, ti1 finA, ti2 finB+exp, ti3 rs/wcol, ti6
    finC), each emitted BEFORE that tile's tree pieces so its small
    DVE ops ride at the queue head and every stage's producers ran a
    full tile earlier: softmax along the free dim of L^T (scalar-engine
    exp with accumulated denominator), folded: out = E^T @ (vbar / s)
    as one fp16 [64]x[64,64] matmul producing the output row directly.

Data-parallel over batch across the 8 NeuronCores (4 batch elements per
core); no collectives needed.  Per-core pipeline: x DMA (~12.9 us/batch
at ~325 GB/s) overlaps compute with xpool bufs=5 (a full batch of
slack decouples each batch's DMA from the PE tail one workload
earlier).  For timing, `unroll` workloads share one For_i body: the
loop back edge is an all-engine barrier + semaphore reset that fully
drains the pipeline (~25us of DMA idle), so it is amortized over 8
workloads, and the finalize deferral chain threads across workloads
inside the body.
"""

import sys

import numpy as np

for _p in ("/opt/trn_rl_repo", "/root/.axon_site/_ro/trn_rl_repo"):
    if _p not in sys.path:
        sys.path.insert(0, _p)

import concourse.bacc as bacc
import concourse.mybir as mybir
import concourse.tile as tile
from concourse import masks
from concourse.bass_utils import run_bass_kernel_spmd

B, C, H, W = 32, 512, 64, 64
N = H * W            # 4096
C8 = 64              # C // 8
NCORES = 8
BPC = B // NCORES    # batch elements per core
NCHUNK = C // 128    # C chunks of 128
TW = 512             # projection tile width (PSUM bank = 512 f32)
NT = N // TW         # 8 projection tiles
NSUB = TW // 128     # transpose subtiles per projection tile
ABUFS = 4            # attpool a_sb buffers (ones cols pre-written per slot)

F32 = mybir.dt.float32
F32R = mybir.dt.float32r
F16 = mybir.dt.float16
AX = mybir.AxisListType.X
MULT = mybir.AluOpType.mult
ADD = mybir.AluOpType.add
ACT_COPY = mybir.ActivationFunctionType.Copy

_NC_CACHE = {}


def _build_nc(loop_n=None, mode="full", unroll=1):
    """Build the bass program.  loop_n wraps the per-batch section in a
    device-side For_i loop (used only for timing: the NEFF then executes the
    whole workload loop_n times back-to-back, making device time measurable
    over the host dispatch overhead); `unroll` workloads are placed in each
    loop body so the all-engine barrier at the back edge amortizes.
    mode: "full" | "dma" (x loads only) | "compute" (batch-0 x loaded once
    outside the loop, engines only)."""
    nc = bacc.Bacc("TRN2", target_bir_lowering=False, debug=False)
    _flags0 = set(mode.split("-")[1:])
    qk_bufs = 2

    x_d = nc.dram_tensor("x", [BPC, C, N], F16, kind="ExternalInput")
    wq_d = nc.dram_tensor("wq", [C8, C], F16, kind="ExternalInput")
    bq_d = nc.dram_tensor("bq", [C8], F32, kind="ExternalInput")
    wk_d = nc.dram_tensor("wk", [C8, C], F16, kind="ExternalInput")
    bk_d = nc.dram_tensor("bk", [C8], F32, kind="ExternalInput")
    wv_d = nc.dram_tensor("wv", [C8, C], F16, kind="ExternalInput")
    bv_d = nc.dram_tensor("bv", [C8], F32, kind="ExternalInput")
    out_d = nc.dram_tensor("out", [BPC, C8], F32, kind="ExternalOutput")

    with tile.TileContext(nc, trace_sim=False) as tc:
        with (
            tc.tile_pool(name="const", bufs=1) as constp,
            tc.tile_pool(name="xpool", bufs=5) as xpool,
            tc.tile_pool(name="attpool", bufs=ABUFS) as attpool,
            tc.tile_pool(name="smallp", bufs=2) as smallp,
            tc.tile_pool(name="ps_qk", bufs=qk_bufs, space="PSUM") as ps_qk,
            tc.tile_pool(name="ps_att", bufs=2, space="PSUM") as ps_att,
            tc.tile_pool(name="ps_small", bufs=1, space="PSUM") as ps_small,
        ):
            # ---------------- one-time prep ----------------
            ident = constp.tile([128, 128], F32)
            masks.make_identity(nc, ident[:])
            ident16 = constp.tile([128, 128], F16)
            nc.scalar.copy(ident16[:], ident[:])

            ones_row = constp.tile([1, C8], F32)
            nc.vector.memset(ones_row[:], 1.0)
            ones2_f32 = constp.tile([128, 2], F32)
            nc.vector.memset(ones2_f32[:], 1.0)
            ones2_16 = constp.tile([128, 2], F16)
            nc.scalar.copy(ones2_16[:], ones2_f32[:])
            # ones at partition 64 (to broadcast the sq row the attention
            # matmul leaves on PSUM partition 64); fp16 so the finalize
            # broadcast matmul runs at 1 cyc/row (fp32 matmuls are 4x and
            # their FP32-HI pass disables FWL for the following matmul)
            ones64 = constp.tile([C8 + 1, C8], F16)
            nc.vector.memset(ones64[C8 : C8 + 1, :], 1.0)

            wq_raw = constp.tile([C8, C], F16)
            nc.sync.dma_start(wq_raw[:], wq_d.ap()[:, :])
            wk_raw = constp.tile([C8, C], F16)
            nc.sync.dma_start(wk_raw[:], wk_d.ap()[:, :])
            wv_raw = constp.tile([C8, C], F16)
            nc.sync.dma_start(wv_raw[:], wv_d.ap()[:, :])

            bq_row = constp.tile([1, C8], F32)
            nc.sync.dma_start(bq_row[:], bq_d.ap().unsqueeze(0))
            bk_row = constp.tile([1, C8], F32)
            nc.sync.dma_start(bk_row[:], bk_d.ap().unsqueeze(0))
            bv_row = constp.tile([1, C8], F32)
            nc.sync.dma_start(bv_row[:], bv_d.ap().unsqueeze(0))

            # fused transposed weight chunks: wqkT[c] = [wqT | wkT] (128
            # cols, FWL-eligible); wvT[c] kept separately -- v is never
            # projected per-pixel, only vbar = wv @ mean_n(x) + bv with the
            # n-sum of x computed on the otherwise-idle DVE/ACT engines
            wqkT = []
            wvT = []
            wqkvT = []
            for c in range(NCHUNK):
                csl = slice(c * 128, (c + 1) * 128)
                pt = ps_small.tile([128, 192], F16, tag="sp")
                nc.tensor.transpose(
                    pt[:, 0:C8], wq_raw[:, csl], ident16[0:C8, 0:C8]
                )
                nc.tensor.transpose(
                    pt[:, C8 : 2 * C8], wk_raw[:, csl], ident16[0:C8, 0:C8]
                )
                nc.tensor.transpose(
                    pt[:, 2 * C8 : 192], wv_raw[:, csl], ident16[0:C8, 0:C8]
                )
                st = constp.tile([128, 128], F16, tag=f"wqkT{c}")
                nc.scalar.copy(st[:], pt[:, 0:128])
                wqkT.append(st)
                sv = constp.tile([128, C8], F16, tag=f"wvT{c}")
                nc.scalar.copy(sv[:], pt[:, 2 * C8 : 192])
                wvT.append(sv)
                if True:
                    s3 = constp.tile([128, 192], F16, tag=f"wqkvT{c}")
                    nc.scalar.copy(s3[:], pt[:, 0:192])
                    wqkvT.append(s3)

            # bias-derived constants
            p_bc = ps_small.tile([C8, C8], F32, tag="sp")
            nc.tensor.matmul(p_bc[:], ones_row[:], bq_row[:], start=True, stop=True)
            bq_bc = constp.tile([C8, C8], F32)  # every row = bq
            nc.scalar.copy(bq_bc[:], p_bc[:])

            p_bk = ps_small.tile([C8, 1], F32, tag="sp")
            nc.tensor.matmul(
                p_bk[:], bk_row[:], ones_row[:, 0:1], start=True, stop=True
            )
            bk_col = constp.tile([C8, 1], F32)
            nc.scalar.copy(bk_col[:], p_bk[:])

            p_bv = ps_small.tile([C8, 1], F32, tag="sp")
            nc.tensor.matmul(
                p_bv[:], bv_row[:], ones_row[:, 0:1], start=True, stop=True
            )
            bv_col = constp.tile([C8, 1], F32)
            nc.scalar.copy(bv_col[:], p_bv[:])

            # throwaway destination for the ACT-engine x-sum reductions
            # (activation needs a main output; only accum_out is consumed)
            act_scratch = constp.tile([128, 1024], F16, tag="act_scratch")

            # pre-write the constant ones columns (0 and 129) into every
            # attpool slot once; the per-tile copies only touch cols 1:129,
            # so the ones persist across the whole loop.  a_sb layout per
            # subtile: [1 | qT (64) | kT (64) | 1] = 130 cols
            for _slot in range(ABUFS):
                a0 = attpool.tile([128, NSUB, 194], F16, tag="a_sb")
                nc.vector.tensor_copy(
                    a0[:, :, 64:130:65],
                    ones2_16[:].unsqueeze(1).broadcast_to([128, NSUB, 2]),
                )

            # ---------------- per batch element ----------------
            multiq = "multiq" in mode.split("-")[1:]
            dma_engs = [nc.sync, nc.sync, nc.scalar, nc.scalar]

            def dma_batch_into(b, pool, tagp):
                # one [128, N] fp16 tile per C-chunk: full 8KB-contiguous
                # rows -> 8KB DMA descriptors (4KB halves measured ~25%
                # slower aggregate under 8-core HBM contention)
                xc = []
                for c in range(NCHUNK):
                    t = pool.tile([128, N], F16, tag=f"{tagp}{c}")
                    eng = dma_engs[c] if multiq else nc.sync
                    eng.dma_start(
                        t[:], x_d.ap()[b, c * 128 : (c + 1) * 128, :]
                    )
                    xc.append(t)
                return xc

            xc_static = None
            if mode.startswith(("compute", "proj")):
                xc_static = dma_batch_into(0, constp, "xs")
            # ablation flags after "-": noxsum (drop x-sum), 2bank (pad
            # qk_ps), dvecopy / splitcopy / act4 (a_sb copy placement)
            flags = set(mode.split("-")[1:])
            no_xsum = "noxsum" in flags
            no_copy = "nocopy" in flags
            vproj = "vproj" in flags
            psum_pad = 256
            copy_eng = (
                "dve" if "dvecopy" in flags
                else "split" if "splitcopy" in flags
                else "act4" if "act4" in flags
                else "act"
            )

            def dma_batch(b):
                return dma_batch_into(b, xpool, "x")

            def emit_batches(flush=None):
                """One workload (BPC batch elements).  Threads the finalize
                deferral chain in and out so consecutive workloads in an
                unrolled loop body pipeline without a serial tail between
                them."""
                if mode == "dma":
                    for b in range(BPC):
                        dma_batch(b)
                    return None
                if mode.startswith(("compute", "proj")):
                    for b in range(BPC):
                        flush = emit_batch(b, xc_static, flush)
                    return flush
                xc_next = dma_batch(0)
                for b in range(BPC):
                    xc_cur = xc_next
                    if b + 1 < BPC:
                        xc_next = dma_batch(b + 1)
                    flush = emit_batch(b, xc_cur, flush)
                return flush

            def final_flush(flush):
                if flush is None:
                    return
                fa = flush[0]()
                fa()()()()

            def emit_attn(att_ps, ti, a_list):
                for s in range(NSUB):
                    first = ti == 0 and s == 0
                    last = ti == NT - 1 and s == NSUB - 1
                    # lhsT=[kT|1] (cols 0:65), rhs=[qT|1|vT] (cols
                    # 65:194) -> out[65,129]: [0:64,0:64]=L0T, [0:64,64]=sk,
                    # [64,0:64]=sq, [64,64]=N, [64,65:129]=sum_n v0 -- the
                    # v-sum rides the same accumulation, so x is never
                    # reduced separately (no DVE tree / GPSIMD folds at all)
                    nc.tensor.matmul(
                        att_ps[:],
                        a_list[:, s, 0 : C8 + 1],
                        a_list[:, s, C8 + 1 : 194],
                        start=first,
                        stop=last,
                    )

            def emit_batch(b, xc, flush_prev):
                pend = []
                finA_prev = [None]
                finB_pending = [None]
                finB2_pending = [None]
                finC_pending = [None]

                # [65, 129]: see emit_attn for the field map
                att_ps = ps_att.tile([C8 + 1, 129], F32)

                # n-sum of x for vbar, split DVE/GPSIMD: two fp16 binary-tree
                # add levels on the DVE (2-byte operands keep the 2x-packed
                # mode; tree partials hold <= 16 x-values, well within fp16
                # range), two more levels on the otherwise-idle GPSIMD, then
                # one small 1x f32 reduce + fp16 cast back on the DVE.  This
                # keeps both the DVE (~40us/iter) and ACT (~32us/iter) under
                # the 51us DMA floor -- the old all-DVE tree was 48us/iter
                # and an ACT accum_out fold pushed ACT to 49us/iter.


                for ti in range(NT):
                    base = ti * TW
                    # x-stationary fused projection: one matmul per
                    # (chunk, subtile) streams [wqT | wkT] through the
                    # stationary x chunk -> [qT | kT] in n-major layout
                    qk_ps = ps_qk.tile([128, NSUB, psum_pad], F32, tag="qk_ps")
                    pw = 192
                    wlist = wqkvT
                    for s in range(NSUB):
                        nsl = slice(base + s * 128, base + (s + 1) * 128)
                        for c in range(NCHUNK):
                            nc.tensor.matmul(
                                qk_ps[:, s, 0:pw],
                                xc[c][:, nsl],
                                wlist[c][:],
                                start=(c == 0),
                                stop=(c == NCHUNK - 1),
                            )

                    # one [128, 4x130] tile holds all 4 subtiles' attention
                    # operands [1 | qT | kT | 1]; a single 3D copy (the ones
                    # columns are pre-written per pool slot)
                    # a_sb copies run on the ACT engine: measured ~0.31 us
                    # per [128,512] psum->f16 copy there vs ~0.9 on DVE,
                    # and it keeps the DVE free for the x-sum tree
                    a_sb = attpool.tile([128, NSUB, 194], F16, tag="a_sb")
                    if no_copy:
                        pass
                    elif copy_eng == "act":
                        # kT then qT on ACT, vT on the now-idle DVE
                        nc.scalar.copy(a_sb[:, :, 0:C8], qk_ps[:, :, C8:128])
                        nc.scalar.copy(a_sb[:, :, C8 + 1 : 129], qk_ps[:, :, 0:C8])
                        nc.vector.tensor_copy(a_sb[:, :, 130:194], qk_ps[:, :, 128:192])
                    elif copy_eng == "act4":
                        for s in range(NSUB):
                            nc.scalar.copy(
                                a_sb[:, s, 1:129], qk_ps[:, s, 0:128]
                            )
                    elif copy_eng == "dve":
                        nc.vector.tensor_copy(a_sb[:, :, 1:129], qk_ps[:, :, 0:128])
                    else:  # split: alternate per tile
                        if ti % 2 == 0:
                            nc.vector.tensor_copy(
                                a_sb[:, :, 1:129], qk_ps[:, :, 0:128]
                            )
                        else:
                            nc.scalar.copy(a_sb[:, :, 1:129], qk_ps[:, :, 0:128])
                    # The previous batch's flush/finalize stages are emitted
                    # BEFORE this tile's xsum pieces: their small DVE ops
                    # then sit AHEAD of the heavy tree adds in the in-order
                    # DVE queue, and each stage's producers ran a full tile
                    # earlier, so no stage ever head-of-line blocks a queue.
                    # Chain: ti0 xfold+attn-flush+sq_sb -> ti1 finA (PE) ->
                    # ti2 finB1 (DVE)+exp (ACT) -> ti3 rs/wcol (DVE) ->
                    # ti6 finC (PE matmul, operands ~3 tiles old).
                    if mode.split("-")[0] != "proj" and flush_prev is not None:
                        if ti == 0:
                            finA_prev[0] = flush_prev[0]()
                        elif ti == 1 and finA_prev[0] is not None:
                            finB_pending[0] = finA_prev[0]()
                        elif ti == 2 and finB_pending[0] is not None:
                            finB2_pending[0] = finB_pending[0]()
                            finB_pending[0] = None
                        elif ti == 3 and finB2_pending[0] is not None:
                            finC_pending[0] = finB2_pending[0]()
                            finB2_pending[0] = None
                        elif ti == 6 and finC_pending[0] is not None:
                            finC_pending[0]()
                            finC_pending[0] = None
                    if mode.split("-")[0] == "proj":
                        continue
                    if len(pend) == 2:
                        emit_attn(*pend.pop(0))
                    pend.append((att_ps, ti, a_sb))

                if mode.split("-")[0] == "proj":
                    return None

                def flush0():
                    # next batch, tile 0: fold this batch's x-sum, flush both
                    # remaining attention tiles (their a_sb copies are >=1
                    # tile old), and copy the sq row; returns the fin_a
                    # handle so the finalize chain staggers one stage per
                    # tile with every producer a full tile ahead
                    emit_attn(*pend[0])
                    emit_attn(*pend[1])
                    return flush_batch(b, att_ps)

                return [flush0]

            def flush_batch(b, att_ps):
                if mode.split("-")[0] == "projattn":
                    return lambda: (lambda: (lambda: (lambda: None)))
                # one partition-64 row copy grabs sq (0:64) and sum_n v0
                # (65:129); fp16 so the fin_a matmuls are fp16 x fp16
                sq_sb = smallp.tile([C8 + 1, 129], F16, tag="sq_sb")
                nc.scalar.copy(sq_sb[C8 : C8 + 1, :], att_ps[C8 : C8 + 1, :])
                return lambda: fin_a(b, att_ps, sq_sb)

            def fin_a(b, att_ps, sq_sb):
                # one PSUM tile holds both finalize matmul outputs:
                # cols 0:64 = sq broadcast to all partitions, col 64 = the
                # attention-side v sum transposed to a column
                fp = ps_small.tile([C8, C8 + 1], F32, tag="sp")
                nc.tensor.matmul(
                    fp[:, C8 : C8 + 1],
                    sq_sb[C8 : C8 + 1, C8 + 1 : 2 * C8 + 1],
                    ones64[C8 : C8 + 1, 0:1],
                    start=True,
                    stop=True,
                )
                nc.tensor.matmul(
                    fp[:, 0:C8],
                    ones64[C8 : C8 + 1, :],
                    sq_sb[C8 : C8 + 1, 0:C8],
                    start=True,
                    stop=True,
                )
                return lambda: fin_b(b, att_ps, fp)

            def fin_b(b, att_ps, fp):
                skp = smallp.tile([C8, 1], F32, tag="skp")
                nc.vector.scalar_tensor_tensor(
                    skp[:], bk_col[:], float(N), att_ps[0:C8, C8 : C8 + 1],
                    op0=MULT, op1=ADD,
                )
                vbar = smallp.tile([C8, 1], F32, tag="vbar")
                nc.vector.scalar_tensor_tensor(
                    vbar[:], fp[:, C8 : C8 + 1], 1.0 / N, bv_col[:],
                    op0=MULT, op1=ADD,
                )
                # LT = L0T + bq_bc * skp + sq_bc * bk
                L1 = smallp.tile([C8, C8], F32, tag="L1")
                nc.vector.scalar_tensor_tensor(
                    L1[:], bq_bc[:], skp[:], att_ps[0:C8, 0:C8],
                    op0=MULT, op1=ADD,
                )
                LT = smallp.tile([C8, C8], F32, tag="LT")
                nc.vector.scalar_tensor_tensor(
                    LT[:], fp[:, 0:C8], bk_col[:], L1[:], op0=MULT, op1=ADD
                )
                # softmax along free dim (the o axis); E and wcol are fp16
                # so the fin_c matmul is a single-pass fp16 matmul (an fp32
                # matmul costs 4 cyc/row and its FP32-HI pass disables FWL
                # for the following projection matmul)
                negm = smallp.tile([C8, 1], F32, tag="negm")
                nc.vector.reduce_max(negm[:], LT[:], axis=AX, negate=True)
                E = smallp.tile([C8, C8], F16, tag="E")
                s_col = smallp.tile([C8, 1], F32, tag="s_col")
                nc.scalar.activation(
                    E[:],
                    LT[:],
                    mybir.ActivationFunctionType.Exp,
                    bias=negm[:],
                    scale=1.0,
                    accum_out=s_col[:],
                )
                return lambda: fin_b2(b, vbar, E, s_col)

            def fin_b2(b, vbar, E, s_col):
                # w = vbar / s ; emitted a few tiles after the exp so these
                # DVE ops never sit in the in-order DVE queue waiting on the
                # ACT round-trip (that wait was blocking the next batch's
                # tree adds behind it, pacing the whole pipeline)
                rs = smallp.tile([C8, 1], F32, tag="rs")
                nc.vector.reciprocal(rs[:], s_col[:])
                wcol = smallp.tile([C8, 1], F16, tag="wcol")
                nc.vector.tensor_tensor(wcol[:], vbar[:], rs[:], op=MULT)
                return lambda: fin_c(b, E, wcol)

            def fin_c(b, E, wcol):
                out_ps = ps_small.tile([1, C8], F32, tag="sp")
                nc.tensor.matmul(out_ps[:], wcol[:], E[:], start=True, stop=True)
                out_row = smallp.tile([1, C8], F32, tag="out_row")
                nc.scalar.copy(out_row[:], out_ps[:])
                nc.gpsimd.dma_start(out_d.ap()[b : b + 1, :], out_row[:])

            if loop_n is None:
                final_flush(emit_batches())
            else:
                hints = (
                    mybir.EngineType.PE,
                    mybir.EngineType.DVE,
                    mybir.EngineType.Activation,
                    mybir.EngineType.SP,
                    mybir.EngineType.Pool,
                )
                assert loop_n % unroll == 0, (loop_n, unroll)
                # unroll>1 places several workloads in one For_i body: the
                # all-engine barrier + sem reset at the loop back edge fully
                # drains the pipeline (x DMA sits idle ~25us while compute
                # finishes), so amortize it over `unroll` workloads; between
                # workloads inside the body the finalize chain threads
                # through emit_batch's deferral slots and the x DMAs of the
                # next workload prefetch during the previous one's compute.
                with tc.For_i(0, loop_n // unroll, 1, hint_engines=hints):
                    flush = None
                    for _ in range(unroll):
                        flush = emit_batches(flush)
                    final_flush(flush)

    nc.compile()
    return nc


def _get_nc(loop_n=None, mode="full", unroll=1):
    key = ("nc", loop_n, mode, unroll)
    if key not in _NC_CACHE:
        _NC_CACHE[key] = _build_nc(loop_n, mode, unroll)
    return _NC_CACHE[key]


def _make_in_maps(x, wq, bq, wk, bk, wv, bv):
    # fp16 shipping: same 10-bit mantissa as the tf32-class device compute,
    # but halves the HBM traffic for x
    xf = np.ascontiguousarray(
        np.asarray(x, dtype=np.float32).reshape(B, C, N).astype(np.float16)
    )
    shared = {
        "wq": np.asarray(wq, np.float32).astype(np.float16),
        "bq": np.asarray(bq, np.float32),
        "wk": np.asarray(wk, np.float32).astype(np.float16),
        "bk": np.asarray(bk, np.float32),
        "wv": np.asarray(wv, np.float32).astype(np.float16),
        "bv": np.asarray(bv, np.float32),
    }
    return [
        {"x": xf[i * BPC : (i + 1) * BPC], **shared} for i in range(NCORES)
    ]


def kernel(x, wq, bq, wk, bk, wv, bv):
    nc = _get_nc()
    in_maps = _make_in_maps(x, wq, bq, wk, bk, wv, bv)
    res = run_bass_kernel_spmd(nc, in_maps, core_ids=list(range(NCORES)))
    out = np.concatenate([res.results[i]["out"] for i in range(NCORES)], axis=0)
    return out.astype(np.float32)



# revision 67
# speedup vs baseline: 1.0141x; 1.0141x over previous
"""Trainium2 Bass kernel for AttentionPooling.

Math (per batch element b):
  xf = x[b] reshaped [C, N] with C=512, N=4096
  q = wq@xf + bq ; k = wk@xf + bk ; v = wv@xf + bv          (each [64, N])
  logits = q @ k^T  [64, 64];  attn = softmax(logits, axis over rows o)
  out[b] = mean_n(attn @ v) = attn @ mean_n(v)              ([64])

Two algebraic collapses make most of the reference FLOPs vanish:
  1. attn does not depend on n, so mean_n(attn @ v) = attn @ vbar with
     vbar = mean_n(v) — the heavy [64, N] attn@v product becomes a [64]
     vector product.
  2. vbar = wv @ mean_n(x) + bv, so v is NEVER projected per-pixel: the
     [64, N] v-projection becomes a free-dim reduction of x (done on the
     otherwise-idle Vector/Scalar engines) plus a tiny [64,512]x[512]
     matmul.  Only q and k are projected per-pixel.

Implementation, per batch element:
  - x, wq, wk, wv are shipped as fp16 (10-bit mantissa, same class as
    tf32/f32r device rounding; empirically 4.5e-3 end-to-end rel err)
    which halves the HBM traffic for x — the dominant memory cost.  x is
    DMAd one [128, 4096] tile per C-chunk: full 8KB-contiguous rows,
    which measure ~25% faster aggregate than 4KB descriptors under
    8-core HBM contention (~323 vs ~260 GB/s per core).
  - X-STATIONARY fused projection: each [128, 128] x chunk-subtile is
    the matmul weight; ONE matmul per (chunk, subtile) streams the fused
    [wqT | wkT] 128-column block (FWL-eligible) through it, producing
    [q0T | k0T] directly in n-major layout (no transposes), fp32 PSUM
    accumulation over the 4 C-chunks.
  - One 3D-AP vector-engine copy per projection tile moves all 4
    subtiles' [1 | qT | kT | 1] attention operands to SBUF (the ones
    columns are pre-written once per pool slot); the ones-augmented
    [65, 65] attention matmuls (lhsT=[kT|1], rhs=[1|qT], accumulated
    over all 32 n-subtiles) produce L0^T AND sum_n k0 (column 0) and
    sum_n q0 (partition-64 row) — the bias-correction sums ride the
    same accumulation.  They run one projection tile behind (and the
    last tile's attention flushes at the NEXT batch's first tile) so
    the in-order PE queue never waits on a DVE copy.
  - sum_n x is split across three engines to stay under the DMA floor:
    two fp16 2x-mode binary-tree add levels on the DVE (tiles 0-3), two
    more levels on the otherwise-idle GPSIMD as pair-fused [128,2,*]
    ops (tiles 2-3), and one small f32 reduce + fp16 cast back on the
    DVE, deferred into the next batch's tile 0 so the slow GPSIMD fold
    never head-of-line blocks the DVE queue.
  - Bias corrections applied analytically on the 64x64 logits:
      L^T = L0^T + bq (x) (sk + N bk) + bk (x) sq
    (valid because logits(q0+bq, k0+bk) is bilinear and attn/softmax only
    needs the full L^T).
  - The finalize chain staggers one stage per tile of the next batch
    (ti0 attn-flush+sq_sb, ti1 finA, ti2 finB+exp, ti3 rs/wcol, ti6
    finC), each emitted BEFORE that tile's tree pieces so its small
    DVE ops ride at the queue head and every stage's producers ran a
    full tile earlier: softmax along the free dim of L^T (scalar-engine
    exp with accumulated denominator), folded: out = E^T @ (vbar / s)
    as one fp16 [64]x[64,64] matmul producing the output row directly.

Data-parallel over batch across the 8 NeuronCores (4 batch elements per
core); no collectives needed.  Per-core pipeline: x DMA (~12.9 us/batch
at ~325 GB/s) overlaps compute with xpool bufs=5 (a full batch of
slack decouples each batch's DMA from the PE tail one workload
earlier).  For timing, `unroll` workloads share one For_i body: the
loop back edge is an all-engine barrier + semaphore reset that fully
drains the pipeline (~25us of DMA idle), so it is amortized over 8
workloads, and the finalize deferral chain threads across workloads
inside the body.
"""

import sys

import numpy as np

for _p in ("/opt/trn_rl_repo", "/root/.axon_site/_ro/trn_rl_repo"):
    if _p not in sys.path:
        sys.path.insert(0, _p)

import concourse.bacc as bacc
import concourse.mybir as mybir
import concourse.tile as tile
from concourse import masks
from concourse.bass_utils import run_bass_kernel_spmd

B, C, H, W = 32, 512, 64, 64
N = H * W            # 4096
C8 = 64              # C // 8
NCORES = 8
BPC = B // NCORES    # batch elements per core
NCHUNK = C // 128    # C chunks of 128
TW = 512             # projection tile width (PSUM bank = 512 f32)
NT = N // TW         # 8 projection tiles
NSUB = TW // 128     # transpose subtiles per projection tile
ABUFS = 4            # attpool a_sb buffers (ones cols pre-written per slot)

F32 = mybir.dt.float32
F32R = mybir.dt.float32r
F16 = mybir.dt.float16
AX = mybir.AxisListType.X
MULT = mybir.AluOpType.mult
ADD = mybir.AluOpType.add
ACT_COPY = mybir.ActivationFunctionType.Copy

_NC_CACHE = {}


def _build_nc(loop_n=None, mode="full", unroll=1):
    """Build the bass program.  loop_n wraps the per-batch section in a
    device-side For_i loop (used only for timing: the NEFF then executes the
    whole workload loop_n times back-to-back, making device time measurable
    over the host dispatch overhead); `unroll` workloads are placed in each
    loop body so the all-engine barrier at the back edge amortizes.
    mode: "full" | "dma" (x loads only) | "compute" (batch-0 x loaded once
    outside the loop, engines only)."""
    nc = bacc.Bacc("TRN2", target_bir_lowering=False, debug=False)
    _flags0 = set(mode.split("-")[1:])
    qk_bufs = 2 if ("qk2" in _flags0 or "vproj" in _flags0) else 4

    x_d = nc.dram_tensor("x", [BPC, C, N], F16, kind="ExternalInput")
    wq_d = nc.dram_tensor("wq", [C8, C], F16, kind="ExternalInput")
    bq_d = nc.dram_tensor("bq", [C8], F32, kind="ExternalInput")
    wk_d = nc.dram_tensor("wk", [C8, C], F16, kind="ExternalInput")
    bk_d = nc.dram_tensor("bk", [C8], F32, kind="ExternalInput")
    wv_d = nc.dram_tensor("wv", [C8, C], F16, kind="ExternalInput")
    bv_d = nc.dram_tensor("bv", [C8], F32, kind="ExternalInput")
    out_d = nc.dram_tensor("out", [BPC, C8], F32, kind="ExternalOutput")

    with tile.TileContext(nc, trace_sim=False) as tc:
        with (
            tc.tile_pool(name="const", bufs=1) as constp,
            tc.tile_pool(name="xpool", bufs=5) as xpool,
            tc.tile_pool(name="attpool", bufs=ABUFS) as attpool,
            tc.tile_pool(name="smallp", bufs=2) as smallp,
            tc.tile_pool(name="ps_qk", bufs=qk_bufs, space="PSUM") as ps_qk,
            tc.tile_pool(name="ps_att", bufs=2, space="PSUM") as ps_att,
            tc.tile_pool(name="ps_small", bufs=1, space="PSUM") as ps_small,
        ):
            # ---------------- one-time prep ----------------
            ident = constp.tile([128, 128], F32)
            masks.make_identity(nc, ident[:])
            ident16 = constp.tile([128, 128], F16)
            nc.scalar.copy(ident16[:], ident[:])

            ones_row = constp.tile([1, C8], F32)
            nc.vector.memset(ones_row[:], 1.0)
            ones2_f32 = constp.tile([128, 2], F32)
            nc.vector.memset(ones2_f32[:], 1.0)
            ones2_16 = constp.tile([128, 2], F16)
            nc.scalar.copy(ones2_16[:], ones2_f32[:])
            # ones at partition 64 (to broadcast the sq row the attention
            # matmul leaves on PSUM partition 64); fp16 so the finalize
            # broadcast matmul runs at 1 cyc/row (fp32 matmuls are 4x and
            # their FP32-HI pass disables FWL for the following matmul)
            ones64 = constp.tile([C8 + 1, C8], F16)
            nc.vector.memset(ones64[C8 : C8 + 1, :], 1.0)

            wq_raw = constp.tile([C8, C], F16)
            nc.sync.dma_start(wq_raw[:], wq_d.ap()[:, :])
            wk_raw = constp.tile([C8, C], F16)
            nc.sync.dma_start(wk_raw[:], wk_d.ap()[:, :])
            wv_raw = constp.tile([C8, C], F16)
            nc.sync.dma_start(wv_raw[:], wv_d.ap()[:, :])

            bq_row = constp.tile([1, C8], F32)
            nc.sync.dma_start(bq_row[:], bq_d.ap().unsqueeze(0))
            bk_row = constp.tile([1, C8], F32)
            nc.sync.dma_start(bk_row[:], bk_d.ap().unsqueeze(0))
            bv_row = constp.tile([1, C8], F32)
            nc.sync.dma_start(bv_row[:], bv_d.ap().unsqueeze(0))

            # fused transposed weight chunks: wqkT[c] = [wqT | wkT] (128
            # cols, FWL-eligible); wvT[c] kept separately -- v is never
            # projected per-pixel, only vbar = wv @ mean_n(x) + bv with the
            # n-sum of x computed on the otherwise-idle DVE/ACT engines
            wqkT = []
            wvT = []
            wqkvT = []
            for c in range(NCHUNK):
                csl = slice(c * 128, (c + 1) * 128)
                pt = ps_small.tile([128, 192], F16, tag="sp")
                nc.tensor.transpose(
                    pt[:, 0:C8], wq_raw[:, csl], ident16[0:C8, 0:C8]
                )
                nc.tensor.transpose(
                    pt[:, C8 : 2 * C8], wk_raw[:, csl], ident16[0:C8, 0:C8]
                )
                nc.tensor.transpose(
                    pt[:, 2 * C8 : 192], wv_raw[:, csl], ident16[0:C8, 0:C8]
                )
                st = constp.tile([128, 128], F16, tag=f"wqkT{c}")
                nc.scalar.copy(st[:], pt[:, 0:128])
                wqkT.append(st)
                sv = constp.tile([128, C8], F16, tag=f"wvT{c}")
                nc.scalar.copy(sv[:], pt[:, 2 * C8 : 192])
                wvT.append(sv)
                if "vproj" in mode.split("-")[1:]:
                    s3 = constp.tile([128, 192], F16, tag=f"wqkvT{c}")
                    nc.scalar.copy(s3[:], pt[:, 0:192])
                    wqkvT.append(s3)

            # bias-derived constants
            p_bc = ps_small.tile([C8, C8], F32, tag="sp")
            nc.tensor.matmul(p_bc[:], ones_row[:], bq_row[:], start=True, stop=True)
            bq_bc = constp.tile([C8, C8], F32)  # every row = bq
            nc.scalar.copy(bq_bc[:], p_bc[:])

            p_bk = ps_small.tile([C8, 1], F32, tag="sp")
            nc.tensor.matmul(
                p_bk[:], bk_row[:], ones_row[:, 0:1], start=True, stop=True
            )
            bk_col = constp.tile([C8, 1], F32)
            nc.scalar.copy(bk_col[:], p_bk[:])

            p_bv = ps_small.tile([C8, 1], F32, tag="sp")
            nc.tensor.matmul(
                p_bv[:], bv_row[:], ones_row[:, 0:1], start=True, stop=True
            )
            bv_col = constp.tile([C8, 1], F32)
            nc.scalar.copy(bv_col[:], p_bv[:])

            # throwaway destination for the ACT-engine x-sum reductions
            # (activation needs a main output; only accum_out is consumed)
            act_scratch = constp.tile([128, 1024], F16, tag="act_scratch")

            # pre-write the constant ones columns (0 and 129) into every
            # attpool slot once; the per-tile copies only touch cols 1:129,
            # so the ones persist across the whole loop.  a_sb layout per
            # subtile: [1 | qT (64) | kT (64) | 1] = 130 cols
            for _slot in range(ABUFS):
                a0 = attpool.tile([128, NSUB, 130], F16, tag="a_sb")
                nc.vector.tensor_copy(
                    a0[:, :, 0:130:129],
                    ones2_16[:].unsqueeze(1).broadcast_to([128, NSUB, 2]),
                )

            # ---------------- per batch element ----------------
            multiq = "multiq" in mode.split("-")[1:]
            dma_engs = [nc.sync, nc.sync, nc.scalar, nc.scalar]

            def dma_batch_into(b, pool, tagp):
                # one [128, N] fp16 tile per C-chunk: full 8KB-contiguous
                # rows -> 8KB DMA descriptors (4KB halves measured ~25%
                # slower aggregate under 8-core HBM contention)
                xc = []
                for c in range(NCHUNK):
                    t = pool.tile([128, N], F16, tag=f"{tagp}{c}")
                    eng = dma_engs[c] if multiq else nc.sync
                    eng.dma_start(
                        t[:], x_d.ap()[b, c * 128 : (c + 1) * 128, :]
                    )
                    xc.append(t)
                return xc

            xc_static = None
            if mode.startswith(("compute", "proj")):
                xc_static = dma_batch_into(0, constp, "xs")
            # ablation flags after "-": noxsum (drop x-sum), 2bank (pad
            # qk_ps), dvecopy / splitcopy / act4 (a_sb copy placement)
            flags = set(mode.split("-")[1:])
            no_xsum = "noxsum" in flags
            no_copy = "nocopy" in flags
            vproj = "vproj" in flags
            psum_pad = 256 if ("2bank" in flags or vproj) else 128
            copy_eng = (
                "dve" if "dvecopy" in flags
                else "split" if "splitcopy" in flags
                else "act4" if "act4" in flags
                else "act"
            )

            def dma_batch(b):
                return dma_batch_into(b, xpool, "x")

            def emit_batches(flush=None):
                """One workload (BPC batch elements).  Threads the finalize
                deferral chain in and out so consecutive workloads in an
                unrolled loop body pipeline without a serial tail between
                them."""
                if mode == "dma":
                    for b in range(BPC):
                        dma_batch(b)
                    return None
                if mode.startswith(("compute", "proj")):
                    for b in range(BPC):
                        flush = emit_batch(b, xc_static, flush)
                    return flush
                xc_next = dma_batch(0)
                for b in range(BPC):
                    xc_cur = xc_next
                    if b + 1 < BPC:
                        xc_next = dma_batch(b + 1)
                    flush = emit_batch(b, xc_cur, flush)
                return flush

            def final_flush(flush):
                if flush is None:
                    return
                fa = flush[0]()
                fa()()()()

            def emit_attn(att_ps, ti, a_list):
                for s in range(NSUB):
                    first = ti == 0 and s == 0
                    last = ti == NT - 1 and s == NSUB - 1
                    # lhsT=[kT|1], rhs=[1|qT] -> out[65,65]:
                    #   [0:64, 0] = sk, [0:64, 1:65] = L0T,
                    #   [64, 1:65] = sq, [64, 0] = N
                    nc.tensor.matmul(
                        att_ps[:],
                        a_list[:, s, C8 + 1 : 2 * C8 + 2],
                        a_list[:, s, 0 : C8 + 1],
                        start=first,
                        stop=last,
                    )

            def emit_batch(b, xc, flush_prev):
                pend = []
                finA_prev = [None]
                finB_pending = [None]
                finB2_pending = [None]
                finC_pending = [None]

                # [65, 65]: [0:64,0]=sk, [0:64,1:65]=L0T, [64,1:65]=sq
                att_ps = ps_att.tile([C8 + 1, C8 + 1], F32)

                # n-sum of x for vbar, split DVE/GPSIMD: two fp16 binary-tree
                # add levels on the DVE (2-byte operands keep the 2x-packed
                # mode; tree partials hold <= 16 x-values, well within fp16
                # range), two more levels on the otherwise-idle GPSIMD, then
                # one small 1x f32 reduce + fp16 cast back on the DVE.  This
                # keeps both the DVE (~40us/iter) and ACT (~32us/iter) under
                # the 51us DMA floor -- the old all-DVE tree was 48us/iter
                # and an ACT accum_out fold pushed ACT to 49us/iter.
                xs16 = smallp.tile([128, NCHUNK], F16, tag="xs16")
                if no_xsum:
                    nc.vector.memset(xs16[:], 0.0)
                    xh2 = xg2 = xg3 = xsa = None
                else:
                    xh2 = smallp.tile([128, NCHUNK, 1024], F16, tag="xh2", bufs=1)
                    xg2 = smallp.tile([128, NCHUNK, 512], F16, tag="xg2", bufs=1)
                    xg3 = smallp.tile([128, NCHUNK, 256], F16, tag="xg3")
                    xsa = smallp.tile([128, NCHUNK], F32, tag="xsa")

                def emit_xsum_piece(k):
                    # k = 0..7 over tiles 0..3 (2 per tile):
                    # k<4: DVE lvl0[k]; k in 4..7: DVE lvl1[k-4]; after the
                    # lvl1 of an odd chunk, the GPSIMD lvl2+lvl3 for that
                    # chunk PAIR as two fused [128,2,*] ops (pair fusion
                    # halves the ~470ns Q7 launch overhead, and starting at
                    # tiles 2-3 instead of 4-5 lets the slow GPSIMD drain
                    # well before the next batch's fold reads it)
                    if k >= 2 * NCHUNK:
                        return
                    if k < NCHUNK:
                        c = k
                        t1 = smallp.tile([128, 2048], F16, tag=f"xt1_{c % 2}", bufs=1)
                        nc.vector.tensor_tensor(
                            t1[:], xc[c][:, 0:2048], xc[c][:, 2048:4096], op=ADD
                        )
                        xtree[c] = t1
                    else:
                        c = k - NCHUNK
                        t1 = xtree[c]
                        nc.vector.tensor_tensor(
                            xh2[:, c, :], t1[:, 0:1024], t1[:, 1024:2048], op=ADD
                        )
                        if c % 2 == 1:
                            p = c - 1
                            nc.gpsimd.tensor_tensor(
                                xg2[:, p : p + 2, :],
                                xh2[:, p : p + 2, 0:512],
                                xh2[:, p : p + 2, 512:1024],
                                op=ADD,
                            )
                            nc.gpsimd.tensor_tensor(
                                xg3[:, p : p + 2, :],
                                xg2[:, p : p + 2, 0:256],
                                xg2[:, p : p + 2, 256:512],
                                op=ADD,
                            )

                def emit_xfold():
                    # final fold: one small f32 reduce over the GPSIMD chunk
                    # partials + fp16 cast.  Deferred into the NEXT batch's
                    # tile 0 (via the flush chain) so the slow GPSIMD folds
                    # have fully drained and the wait never head-of-line
                    # blocks this batch's DVE tree in the in-order queue.
                    nc.vector.tensor_reduce(
                        xsa[:, :], xg3[:, :, :], axis=AX, op=ADD
                    )
                    nc.vector.tensor_copy(xs16[:], xsa[:])

                xtree = [None] * NCHUNK

                for ti in range(NT):
                    base = ti * TW
                    # x-stationary fused projection: one matmul per
                    # (chunk, subtile) streams [wqT | wkT] through the
                    # stationary x chunk -> [qT | kT] in n-major layout
                    qk_ps = ps_qk.tile([128, NSUB, psum_pad], F32, tag="qk_ps")
                    pw = 192 if vproj else 128
                    wlist = wqkvT if vproj else wqkT
                    for s in range(NSUB):
                        nsl = slice(base + s * 128, base + (s + 1) * 128)
                        for c in range(NCHUNK):
                            nc.tensor.matmul(
                                qk_ps[:, s, 0:pw],
                                xc[c][:, nsl],
                                wlist[c][:],
                                start=(c == 0),
                                stop=(c == NCHUNK - 1),
                            )

                    # one [128, 4x130] tile holds all 4 subtiles' attention
                    # operands [1 | qT | kT | 1]; a single 3D copy (the ones
                    # columns are pre-written per pool slot)
                    # a_sb copies run on the ACT engine: measured ~0.31 us
                    # per [128,512] psum->f16 copy there vs ~0.9 on DVE,
                    # and it keeps the DVE free for the x-sum tree
                    a_sb = attpool.tile([128, NSUB, 130], F16, tag="a_sb")
                    if no_copy:
                        pass
                    elif copy_eng == "act":
                        nc.scalar.copy(a_sb[:, :, 1:129], qk_ps[:, :, 0:128])
                    elif copy_eng == "act4":
                        for s in range(NSUB):
                            nc.scalar.copy(
                                a_sb[:, s, 1:129], qk_ps[:, s, 0:128]
                            )
                    elif copy_eng == "dve":
                        nc.vector.tensor_copy(a_sb[:, :, 1:129], qk_ps[:, :, 0:128])
                    else:  # split: alternate per tile
                        if ti % 2 == 0:
                            nc.vector.tensor_copy(
                                a_sb[:, :, 1:129], qk_ps[:, :, 0:128]
                            )
                        else:
                            nc.scalar.copy(a_sb[:, :, 1:129], qk_ps[:, :, 0:128])
                    # The previous batch's flush/finalize stages are emitted
                    # BEFORE this tile's xsum pieces: their small DVE ops
                    # then sit AHEAD of the heavy tree adds in the in-order
                    # DVE queue, and each stage's producers ran a full tile
                    # earlier, so no stage ever head-of-line blocks a queue.
                    # Chain: ti0 xfold+attn-flush+sq_sb -> ti1 finA (PE) ->
                    # ti2 finB1 (DVE)+exp (ACT) -> ti3 rs/wcol (DVE) ->
                    # ti6 finC (PE matmul, operands ~3 tiles old).
                    if mode.split("-")[0] != "proj" and flush_prev is not None:
                        if ti == 0:
                            finA_prev[0] = flush_prev[0]()
                        elif ti == 1 and finA_prev[0] is not None:
                            finB_pending[0] = finA_prev[0]()
                        elif ti == 2 and finB_pending[0] is not None:
                            finB2_pending[0] = finB_pending[0]()
                            finB_pending[0] = None
                        elif ti == 3 and finB2_pending[0] is not None:
                            finC_pending[0] = finB2_pending[0]()
                            finB2_pending[0] = None
                        elif ti == 6 and finC_pending[0] is not None:
                            finC_pending[0]()
                            finC_pending[0] = None
                    # ~2 tree pieces per tile keep the DVE fed without any
                    # long op delaying downstream consumers
                    if not no_xsum:
                        emit_xsum_piece(2 * ti)
                        emit_xsum_piece(2 * ti + 1)
                    if mode.split("-")[0] == "proj":
                        if not no_xsum and ti == NT - 1:
                            emit_xfold()
                        continue
                    if len(pend) == 2:
                        emit_attn(*pend.pop(0))
                    pend.append((att_ps, ti, a_sb))

                if mode.split("-")[0] == "proj":
                    return None

                def flush0():
                    # next batch, tile 0: fold this batch's x-sum, flush both
                    # remaining attention tiles (their a_sb copies are >=1
                    # tile old), and copy the sq row; returns the fin_a
                    # handle so the finalize chain staggers one stage per
                    # tile with every producer a full tile ahead
                    if not no_xsum:
                        emit_xfold()
                    emit_attn(*pend[0])
                    emit_attn(*pend[1])
                    return flush_batch(b, att_ps, xs16)

                return [flush0]

            def flush_batch(b, att_ps, xs16):
                if mode.split("-")[0] == "projattn":
                    return lambda: (lambda: (lambda: (lambda: None)))
                # fp16 so the broadcast matmul in fin_a is fp16 x fp16
                sq_sb = smallp.tile([C8 + 1, C8], F16, tag="sq_sb")
                nc.scalar.copy(sq_sb[C8 : C8 + 1, :], att_ps[C8 : C8 + 1, 1 : C8 + 1])
                return lambda: fin_a(b, att_ps, sq_sb, xs16)

            def fin_a(b, att_ps, sq_sb, xs16):
                # one PSUM tile holds both finalize matmul outputs:
                # cols 0:64 = sq broadcast to all partitions, col 64 = the
                # v sum column = wv @ (sum_n x), from the engine-side x-sums
                fp = ps_small.tile([C8, C8 + 1], F32, tag="sp")
                for c in range(NCHUNK):
                    nc.tensor.matmul(
                        fp[:, C8 : C8 + 1],
                        wvT[c][:],
                        xs16[:, c : c + 1],
                        start=(c == 0),
                        stop=(c == NCHUNK - 1),
                    )
                nc.tensor.matmul(
                    fp[:, 0:C8],
                    ones64[C8 : C8 + 1, :],
                    sq_sb[C8 : C8 + 1, :],
                    start=True,
                    stop=True,
                )
                return lambda: fin_b(b, att_ps, fp)

            def fin_b(b, att_ps, fp):
                skp = smallp.tile([C8, 1], F32, tag="skp")
                nc.vector.scalar_tensor_tensor(
                    skp[:], bk_col[:], float(N), att_ps[0:C8, 0:1], op0=MULT, op1=ADD
                )
                vbar = smallp.tile([C8, 1], F32, tag="vbar")
                nc.vector.scalar_tensor_tensor(
                    vbar[:], fp[:, C8 : C8 + 1], 1.0 / N, bv_col[:],
                    op0=MULT, op1=ADD,
                )
                # LT = L0T + bq_bc * skp + sq_bc * bk
                L1 = smallp.tile([C8, C8], F32, tag="L1")
                nc.vector.scalar_tensor_tensor(
                    L1[:], bq_bc[:], skp[:], att_ps[0:C8, 1 : C8 + 1],
                    op0=MULT, op1=ADD,
                )
                LT = smallp.tile([C8, C8], F32, tag="LT")
                nc.vector.scalar_tensor_tensor(
                    LT[:], fp[:, 0:C8], bk_col[:], L1[:], op0=MULT, op1=ADD
                )
                # softmax along free dim (the o axis); E and wcol are fp16
                # so the fin_c matmul is a single-pass fp16 matmul (an fp32
                # matmul costs 4 cyc/row and its FP32-HI pass disables FWL
                # for the following projection matmul)
                negm = smallp.tile([C8, 1], F32, tag="negm")
                nc.vector.reduce_max(negm[:], LT[:], axis=AX, negate=True)
                E = smallp.tile([C8, C8], F16, tag="E")
                s_col = smallp.tile([C8, 1], F32, tag="s_col")
                nc.scalar.activation(
                    E[:],
                    LT[:],
                    mybir.ActivationFunctionType.Exp,
                    bias=negm[:],
                    scale=1.0,
                    accum_out=s_col[:],
                )
                return lambda: fin_b2(b, vbar, E, s_col)

            def fin_b2(b, vbar, E, s_col):
                # w = vbar / s ; emitted a few tiles after the exp so these
                # DVE ops never sit in the in-order DVE queue waiting on the
                # ACT round-trip (that wait was blocking the next batch's
                # tree adds behind it, pacing the whole pipeline)
                rs = smallp.tile([C8, 1], F32, tag="rs")
                nc.vector.reciprocal(rs[:], s_col[:])
                wcol = smallp.tile([C8, 1], F16, tag="wcol")
                nc.vector.tensor_tensor(wcol[:], vbar[:], rs[:], op=MULT)
                return lambda: fin_c(b, E, wcol)

            def fin_c(b, E, wcol):
                out_ps = ps_small.tile([1, C8], F32, tag="sp")
                nc.tensor.matmul(out_ps[:], wcol[:], E[:], start=True, stop=True)
                out_row = smallp.tile([1, C8], F32, tag="out_row")
                nc.scalar.copy(out_row[:], out_ps[:])
                nc.gpsimd.dma_start(out_d.ap()[b : b + 1, :], out_row[:])

            if loop_n is None:
                final_flush(emit_batches())
            else:
                hints = (
                    mybir.EngineType.PE,
                    mybir.EngineType.DVE,
                    mybir.EngineType.Activation,
                    mybir.EngineType.SP,
                    mybir.EngineType.Pool,
                )
                assert loop_n % unroll == 0, (loop_n, unroll)
                # unroll>1 places several workloads in one For_i body: the
                # all-engine barrier + sem reset at the loop back edge fully
                # drains the pipeline (x DMA sits idle ~25us while compute
                # finishes), so amortize it over `unroll` workloads; between
                # workloads inside the body the finalize chain threads
                # through emit_batch's deferral slots and the x DMAs of the
                # next workload prefetch during the previous one's compute.
                with tc.For_i(0, loop_n // unroll, 1, hint_engines=hints):
                    flush = None
                    for _ in range(unroll):
                        flush = emit_batches(flush)
                    final_flush(flush)

    nc.compile()
    return nc


def _get_nc(loop_n=None, mode="full", unroll=1):
    key = ("nc", loop_n, mode, unroll)
    if key not in _NC_CACHE:
        _NC_CACHE[key] = _build_nc(loop_n, mode, unroll)
    return _NC_CACHE[key]


def _make_in_maps(x, wq, bq, wk, bk, wv, bv):
    # fp16 shipping: same 10-bit mantissa as the tf32-class device compute,
    # but halves the HBM traffic for x
    xf = np.ascontiguousarray(
        np.asarray(x, dtype=np.float32).reshape(B, C, N).astype(np.float16)
    )
    shared = {
        "wq": np.asarray(wq, np.float32).astype(np.float16),
        "bq": np.asarray(bq, np.float32),
        "wk": np.asarray(wk, np.float32).astype(np.float16),
        "bk": np.asarray(bk, np.float32),
        "wv": np.asarray(wv, np.float32).astype(np.float16),
        "bv": np.asarray(bv, np.float32),
    }
    return [
        {"x": xf[i * BPC : (i + 1) * BPC], **shared} for i in range(NCORES)
    ]


def kernel(x, wq, bq, wk, bk, wv, bv):
    nc = _get_nc()
    in_maps = _make_in_maps(x, wq, bq, wk, bk, wv, bv)
    res = run_bass_kernel_spmd(nc, in_maps, core_ids=list(range(NCORES)))
    out = np.concatenate([res.results[i]["out"] for i in range(NCORES)], axis=0)
    return out.astype(np.float32)

